# revision 27
# baseline (speedup 1.0000x reference)
"""Bass/Trainium2 kernel for nn_BoundaryLoss (8-core data-parallel), v3.

loss = mean( w * ce ) over (B=16, H=360, W=640), where
  ce = logsumexp_c(pred) - pred[target]   (C=7)
  w  = 10 if the 17-tap ellipse window around the pixel is NOT constant
       (cv2 border semantics = replicate clamp here), else 1.

Per core: 2 images, rows in 3 groups (124/124/112); both images ride
side-by-side in the free dim so every engine op covers 2 images.

Per row-group (R rows):
  t16 [nin,2*644] fp16 target + halo rows + replicate pads   (cast DMA)
  P   [R,2*4480]  fp16 pred (one DMA per image)              (cast DMA)
  E = exp(P-1) -> fp8                                        (Act)
  boundary: ONE integer-hash 17-tap conv (random signed prime weights,
    center extra -sum(w)); window constant => hash == 0 exactly (integer
    arithmetic, exact in fp16 weights / f32 PSUM).  Non-constant windows
    hash to 0 with p ~ 1e-3-1e-4 -> ~4e-4 relative loss shift (tol 2e-2).
  m_inv = (hash == 0)  (Pool ts per image, accum -> Sum m_inv)
  S = sum_c E  (fp8 DoubleRow c-pair matmuls + single c=6)
  lse = Ln(S)  (Act)  [= logsumexp - 1]
  M: 7x tensor_scalar is_equal (DVE 4x mode); MP = M*P in place (2x tt)
  PK = sum_c MP  (PE fp16 identity matmuls)  [= pred[target]]
  d  = lse - PK  (Pool stt, accum -> Sum d)
  md = m_inv * d (DVE stt, accum -> Sum m_inv*d)
Host: loss = sum_cores[10*Sd - 9*Smd + 10*N_core - 9*Sm_inv] / N_total
      (+N folds the exp(x-1) shift: ce = d + 1.)
"""

import sys

for _p in ("/opt/trn_rl_repo",):
    if _p not in sys.path:
        sys.path.insert(0, _p)

import numpy as np
import ml_dtypes

import bass_rust
import concourse.bass as bass
import concourse.mybir as mybir
from concourse.ap import AP as APClass
from concourse.tile import TileContext
from concourse import bass_utils

F32 = mybir.dt.float32
F16 = mybir.dt.float16
F8 = mybir.dt.float8e4
I32 = mybir.dt.int32
PM = mybir.MatmulPerfMode

B_PER_CORE = 2
H, W, C = 360, 640, 7
GROUPS = [(0, 124, 0), (124, 124, 1), (248, 112, 2)]  # (row0, rows, variant)
WP = W + 4
NACC = 24

VERT = {0: [-2, -1, 0, 1, 2], -1: [-1, 0, 1], 1: [-1, 0, 1],
        -2: [-1, 0, 1], 2: [-1, 0, 1]}
TAPS = [(dy, dx) for dx, dys in VERT.items() for dy in dys]
DXS = [-2, -1, 0, 1, 2]
BANDW = 124
# cw16 layout: 15 conv bands (variant x dx) + identity
CW16_BLOCKS = len(GROUPS) * len(DXS) + 1
CW16_COLS = CW16_BLOCKS * BANDW


def _build_convw():
    """cw16 [128, CW16_COLS] fp16: hash-conv bands + identity (PK matmuls).
    cw8 [128, 2, 124] fp8: DoubleRow identity pair (S matmuls)."""
    rng = np.random.default_rng(20260808)
    primes = np.array([3, 7, 11, 19, 23, 31, 43, 53], dtype=np.int64)
    w = {t: int(primes[rng.integers(0, len(primes))]) * int(rng.choice([-1, 1]))
         for t in TAPS}
    wc = dict(w)
    wc[(0, 0)] = w[(0, 0)] - sum(w.values())

    w16 = np.zeros((128, CW16_BLOCKS, BANDW), dtype=np.float32)
    for vi, (r0, R, _v) in enumerate(GROUPS):
        in_r0 = max(r0 - 2, 0)
        for di, dx in enumerate(DXS):
            blk = vi * len(DXS) + di
            for j in range(R):
                for dy in VERT[dx]:
                    rr = min(max(r0 + j + dy, 0), H - 1)
                    w16[rr - in_r0, blk, j] += wc[(dy, dx)]
    for k in range(BANDW):
        w16[k, CW16_BLOCKS - 1, k] = 1.0
    assert np.abs(w16).max() <= 2048, "fp16 integer exactness"
    cw16 = np.ascontiguousarray(
        w16.reshape(128, CW16_COLS).astype(np.float16))

    w8 = np.zeros((128, 2, BANDW), dtype=np.float32)
    for k in range(BANDW):
        w8[k, 0, k] = 1.0
        w8[k, 1, k] = 1.0
    cw8 = np.ascontiguousarray(
        w8.reshape(128, 2 * BANDW).astype(ml_dtypes.float8_e4m3fn))
    return cw16, cw8


def split_multiwait_drains(nc, max_waits=1):
    """This walrus build rejects >1 sync-waits on CTRL-class instructions
    (the Tile end-of-kernel drain).  Split extra waits into preceding
    single-wait EventSemaphore instructions on the same engine."""
    fn = nc.m.functions[0]
    for bb in fn.blocks:
        for inst in list(bb.instructions):
            si = inst.sync_info
            if si is None or len(si.on_wait) <= max_waits:
                continue
            waits = list(si.on_wait)
            keep, extra = waits[:max_waits], waits[max_waits:]
            new_insts = []
            for k, wt in enumerate(extra):
                es = mybir.InstEventSemaphore(
                    name=f"{inst.name}-waitsplit-{k}", ins=[], outs=[])
                es.engine = inst.engine
                es.sync_info = bass_rust.SyncInfo(on_wait=[wt], on_update=[])
                nc.register_instruction(es, overwrite=True)
                new_insts.append(es)
            inst.sync_info = bass_rust.SyncInfo(
                on_wait=keep, on_update=list(si.on_update))
            pos = [i.name for i in bb.instructions].index(inst.name)
            for k, es in enumerate(new_insts):
                bb.instructions.insert(pos + k, es)


def _chunks(lo, hi, step=512):
    out = []
    while lo < hi:
        nxt = min(hi, (lo // step + 1) * step)
        out.append((lo, nxt))
        lo = nxt
    return out


def _pair_view(v2d, stride):
    """[P, cn] contiguous 2-D AP -> [P, 2, cn] with the two tiles `stride`
    elements apart (DoubleRow rhs)."""
    ap = [list(p) for p in v2d.ap]
    assert len(ap) == 2, ap
    return APClass(tensor=v2d.tensor, offset=v2d.offset,
                   ap=[ap[0], [stride, 2], ap[1]])


class _Dg:
    def __init__(self, gi):
        self.gi = gi
        self.r0, self.R, self.var = GROUPS[gi]
        self.in_r0 = max(self.r0 - 2, 0)
        self.in_r1 = min(self.r0 + self.R + 2, H)
        self.n_in = self.in_r1 - self.in_r0
        self.po = self.r0 - self.in_r0  # partition offset of center rows


def emit_loads(nc, io, sm, aps, dg):
    pred, target = aps["pred"], aps["target"]
    # P first: it gates exp, the critical chain.  One DMA per image so the
    # first image's exp can start while the second streams.
    dg.P = io.tile([128, 2 * C * W], F16, tag="P")
    for b in range(2):
        nc.gpsimd.dma_start(
            out=dg.P[:dg.R, b * C * W:(b + 1) * C * W].rearrange(
                "p (c w) -> p c w", c=C),
            in_=pred[b, :, dg.r0:dg.r0 + dg.R, :].rearrange("c r w -> r c w"))
    dg.t16 = sm.tile([128, 2 * WP], F16, tag="t16")
    nc.gpsimd.dma_start(
        out=dg.t16[:dg.n_in, :].rearrange(
            "p (b wp) -> p b wp", b=2)[:, :, 2:2 + W],
        in_=target[:, dg.in_r0:dg.in_r1, :].rearrange("b r w -> r b w"))


def emit_head(nc, pools, aps, dg):
    io, sm, cvp, spp = pools
    cw16, cw8 = aps["cw16"], aps["cw8"]
    alu = mybir.AluOpType
    AF = mybir.ActivationFunctionType
    R, n_in, gi, po = dg.R, dg.n_in, dg.gi, dg.po

    # horizontal replicate pads (Pool, tiny)
    t16v = dg.t16.rearrange("p (b wp) -> p b wp", b=2)
    for b in range(2):
        nc.gpsimd.tensor_copy(t16v[:n_in, b, 0:2],
                              t16v[:n_in, b, 2:3].broadcast_to([n_in, 2]))
        nc.gpsimd.tensor_copy(t16v[:n_in, b, W + 2:W + 4],
                              t16v[:n_in, b, W + 1:W + 2].broadcast_to([n_in, 2]))

    # E = exp(P - 1) -> fp8 (dg0: per-image ops to chase the P DMA halves)
    dg.E = io.tile([128, 2 * C * W], F8, tag="E")
    splits = ((0, C * W), (C * W, 2 * C * W)) if gi == 0 else ((0, 2 * C * W),)
    for (s0, s1) in splits:
        nc.scalar.activation(dg.E[:R, s0:s1], dg.P[:R, s0:s1], AF.Exp,
                             bias=aps["neg1"][:R, 0:1], scale=1.0)

    # hash conv (PE fp16) + m_inv, per image
    cw16v = cw16.rearrange("p (blk j) -> p blk j", blk=CW16_BLOCKS)
    dg.m_inv = sm.tile([128, 2 * W], F16, tag="m_inv")
    for b in range(2):
        rr = cvp.tile([128, W], F32, tag="rr")
        for (o0, o1) in _chunks(0, W):
            cn = o1 - o0
            for di, dx in enumerate(DXS):
                blk = dg.var * len(DXS) + di
                col = b * WP + 2 + dx + o0
                nc.tensor.matmul(rr[:R, o0:o1], cw16v[:n_in, blk, :R],
                                 dg.t16[:n_in, col:col + cn],
                                 start=(di == 0), stop=(di == len(DXS) - 1))
        nc.gpsimd.tensor_scalar(
            out=dg.m_inv[:R, b * W:(b + 1) * W], in0=rr[:R, :],
            scalar1=0.0, scalar2=None, op0=alu.is_equal,
            accum_out=aps["a_mi"][:R, 2 * gi + b:2 * gi + b + 1])

    # masks (7x ts is_equal, 4x mode), then MP = M*P in place
    dg.M = io.tile([128, 2 * C * W], F16, tag="M")
    Mv = dg.M.rearrange("p (b c w) -> p b c w", b=2, c=C)
    tc16 = t16v[po:po + R, :, 2:2 + W]
    for c in range(C):
        nc.vector.tensor_scalar(out=Mv[:R, :, c, :], in0=tc16,
                                scalar1=float(c), scalar2=None, op0=alu.is_equal)
    # MP = M*P in place, one op per image so PK/d/md pipeline per image
    for b in range(2):
        s = slice(b * C * W, (b + 1) * C * W)
        nc.vector.tensor_tensor(out=dg.M[:R, s], in0=dg.M[:R, s],
                                in1=dg.P[:R, s], op=alu.mult)

    # S = sum_c E (fp8 DR pairs + single c=6); PK = sum_c MP (fp16)
    dg.S = spp.tile([128, 2 * W], F32, tag="S")
    dg.PK = spp.tile([128, 2 * W], F32, tag="PK")
    cw8v = cw8.rearrange("p (two j) -> p two j", two=2)
    id2 = cw8v[:R, :, :R]
    id8 = cw8v[:R, 0, :R]
    id16 = cw16v[:R, CW16_BLOCKS - 1, :R]
    def _emit_pk():
        for b in range(2):
            for (o0, o1) in _chunks(b * W, (b + 1) * W):
                rel0 = o0 - b * W
                cn = o1 - o0
                for c in range(C):
                    col = b * C * W + c * W + rel0
                    nc.tensor.matmul(dg.PK[:R, o0:o1], id16,
                                     dg.M[:R, col:col + cn],
                                     start=(c == 0), stop=(c == C - 1))

    def _emit_s():
        for b in range(2):
            for (o0, o1) in _chunks(b * W, (b + 1) * W):
                rel0 = o0 - b * W
                cn = o1 - o0
                for ci in range(3):
                    col = b * C * W + (2 * ci) * W + rel0
                    rhs = _pair_view(dg.E[:R, col:col + cn], W)
                    nc.tensor.matmul(dg.S[:R, o0:o1], id2, rhs,
                                     start=(ci == 0), stop=False,
                                     perf_mode=PM.DoubleRow)
                col = b * C * W + 6 * W + rel0
                nc.tensor.matmul(dg.S[:R, o0:o1], id8, dg.E[:R, col:col + cn],
                                 start=False, stop=True)

    # On the last group close the d->md tail as early as possible: MP is
    # ready before E there, so PK goes first.  Earlier groups: S first so
    # ln/exp pipelining on Act is not blocked behind MP.
    if gi == len(GROUPS) - 1:
        _emit_pk()
        _emit_s()
    else:
        _emit_s()
        _emit_pk()


def emit_tail(nc, pools, aps, dg):
    io, sm, cvp, spp = pools
    alu = mybir.AluOpType
    AF = mybir.ActivationFunctionType
    R, gi = dg.R, dg.gi

    last = gi == len(GROUPS) - 1
    lse = sm.tile([128, 2 * W], F16, tag="lse")
    nc.scalar.activation(lse[:R, :], dg.S[:R, :], AF.Ln)
    d = sm.tile([128, 2 * W], F16, tag="d")
    mdj = sm.tile([128, 2 * W], F16, tag="mdj")
    for b in range(2):
        s = slice(b * W, (b + 1) * W)
        nc.gpsimd.scalar_tensor_tensor(
            out=d[:R, s], in0=dg.PK[:R, s], scalar=-1.0, in1=lse[:R, s],
            op0=alu.mult, op1=alu.add,
            accum_out=aps["a_d"][:R, 2 * gi + b:2 * gi + b + 1])
        # md on DVE only where it closes the kernel; else keep DVE light
        eng = nc.vector if last else nc.gpsimd
        eng.scalar_tensor_tensor(
            out=mdj[:R, s], in0=dg.m_inv[:R, s], scalar=0.0, in1=d[:R, s],
            op0=alu.bypass, op1=alu.mult,
            accum_out=aps["a_md"][:R, 2 * gi + b:2 * gi + b + 1])


def build_nc(io_bufs=2, sm_bufs=2):
    nc = bass.Bass()
    pred = nc.dram_tensor("pred", [B_PER_CORE, C, H, W], F32,
                          kind="ExternalInput")
    target = nc.dram_tensor("target", [B_PER_CORE, H, W], I32,
                            kind="ExternalInput")
    convw16 = nc.dram_tensor("convw16", [128, CW16_COLS], F16,
                             kind="ExternalInput")
    convw8 = nc.dram_tensor("convw8", [128, 2 * BANDW], F8,
                            kind="ExternalInput")
    acc_out = nc.dram_tensor("acc", [128, NACC], F32, kind="ExternalOutput")

    with TileContext(nc, pool_alloc_mode="stack") as tc:
        with (
            tc.tile_pool(name="io", bufs=io_bufs) as io,
            tc.tile_pool(name="sm", bufs=sm_bufs) as sm,
            tc.tile_pool(name="cv", bufs=1, space="PSUM") as cvp,
            tc.tile_pool(name="sp", bufs=1, space="PSUM") as spp,
            tc.tile_pool(name="const", bufs=1) as cpool,
        ):
            a_mi = cpool.tile([128, 8], F32)
            nc.vector.memset(a_mi[:, :], 0.0)
            a_d = cpool.tile([128, 8], F32)
            nc.vector.memset(a_d[:, :], 0.0)
            a_md = cpool.tile([128, 8], F32)
            nc.vector.memset(a_md[:, :], 0.0)
            neg1 = cpool.tile([128, 1], F32)
            nc.vector.memset(neg1[:, :], -1.0)
            cw16_sb = cpool.tile([128, CW16_COLS], F16)
            cw8_sb = cpool.tile([128, 2 * BANDW], F8)
            aps = {"pred": pred.ap(), "target": target.ap(),
                   "cw16": cw16_sb, "cw8": cw8_sb, "neg1": neg1,
                   "a_mi": a_mi, "a_d": a_d, "a_md": a_md}
            pools = (io, sm, cvp, spp)

            dgs = [_Dg(i) for i in range(len(GROUPS))]
            emit_loads(nc, io, sm, aps, dgs[0])
            # weights after the first P/t16 loads, split per variant block so
            # no single transfer delays P on the shared DMA engines
            nbv = len(DXS) * BANDW
            nc.sync.dma_start(out=cw8_sb[:, :], in_=convw8.ap())
            nc.sync.dma_start(out=cw16_sb[:, 15 * BANDW:],
                              in_=convw16.ap()[:, 15 * BANDW:])
            for vi in range(len(GROUPS)):
                nc.sync.dma_start(out=cw16_sb[:, vi * nbv:(vi + 1) * nbv],
                                  in_=convw16.ap()[:, vi * nbv:(vi + 1) * nbv])
            emit_loads(nc, io, sm, aps, dgs[1])
            for g in range(len(dgs)):
                emit_head(nc, pools, aps, dgs[g])
                if g + 2 < len(dgs):
                    emit_loads(nc, io, sm, aps, dgs[g + 2])
                if g > 0:
                    emit_tail(nc, pools, aps, dgs[g - 1])
            emit_tail(nc, pools, aps, dgs[-1])

            nc.sync.dma_start(out=acc_out.ap()[:, 0:8], in_=a_mi[:, :])
            nc.sync.dma_start(out=acc_out.ap()[:, 8:16], in_=a_d[:, :])
            nc.sync.dma_start(out=acc_out.ap()[:, 16:24], in_=a_md[:, :])

    split_multiwait_drains(nc)
    return nc


_CACHED = {}


def _get_nc():
    if "nc" not in _CACHED:
        _CACHED["nc"] = build_nc()
        _CACHED["convw16"], _CACHED["convw8"] = _build_convw()
    return _CACHED["nc"], _CACHED["convw16"], _CACHED["convw8"]


def combine_acc(acc_tiles):
    n_core = B_PER_CORE * H * W
    total = 0.0
    for a in acc_tiles:
        a = a.astype(np.float64)
        smi = a[:, 0:8].sum()
        sd = a[:, 8:16].sum()
        smd = a[:, 16:24].sum()
        total += 10.0 * sd - 9.0 * smd + 10.0 * n_core - 9.0 * smi
    return np.float32(total / (8 * n_core))


def kernel(pred, target):
    nc, convw16, convw8 = _get_nc()
    n_cores = 8
    in_maps = []
    for i in range(n_cores):
        in_maps.append({
            "pred": np.ascontiguousarray(pred[2 * i:2 * i + 2]),
            "target": np.ascontiguousarray(target[2 * i:2 * i + 2]),
            "convw16": convw16,
            "convw8": convw8,
        })
    res = bass_utils.run_bass_kernel_spmd(nc, in_maps,
                                          core_ids=list(range(n_cores)))
    return combine_acc([r["acc"] for r in res.results])


# revision 29
# speedup vs baseline: 1.0130x; 1.0130x over previous
"""Bass/Trainium2 kernel for nn_BoundaryLoss (8-core data-parallel), v3.

loss = mean( w * ce ) over (B=16, H=360, W=640), where
  ce = logsumexp_c(pred) - pred[target]   (C=7)
  w  = 10 if the 17-tap ellipse window around the pixel is NOT constant
       (cv2 border semantics = replicate clamp here), else 1.

Per core: 2 images, rows in 3 groups (124/124/112); both images ride
side-by-side in the free dim so every engine op covers 2 images.

Per row-group (R rows):
  t16 [nin,2*644] fp16 target + halo rows + replicate pads   (cast DMA)
  P   [R,2*4480]  fp16 pred (one DMA per image)              (cast DMA)
  E = exp(P-1) -> fp8                                        (Act)
  boundary: ONE integer-hash 17-tap conv (random signed prime weights,
    center extra -sum(w)); window constant => hash == 0 exactly (integer
    arithmetic, exact in fp16 weights / f32 PSUM).  Non-constant windows
    hash to 0 with p ~ 1e-3-1e-4 -> ~4e-4 relative loss shift (tol 2e-2).
  m_inv = (hash == 0)  (Pool ts per image, accum -> Sum m_inv)
  S = sum_c E  (fp8 DoubleRow c-pair matmuls + single c=6)
  lse = Ln(S)  (Act)  [= logsumexp - 1]
  M: 7x tensor_scalar is_equal (DVE 4x mode); MP = M*P in place (2x tt)
  PK = sum_c MP  (PE fp16 identity matmuls)  [= pred[target]]
  d  = lse - PK  (Pool stt, accum -> Sum d)
  md = m_inv * d (DVE stt, accum -> Sum m_inv*d)
Host: loss = sum_cores[10*Sd - 9*Smd + 10*N_core - 9*Sm_inv] / N_total
      (+N folds the exp(x-1) shift: ce = d + 1.)
"""

import sys

for _p in ("/opt/trn_rl_repo",):
    if _p not in sys.path:
        sys.path.insert(0, _p)

import numpy as np
import ml_dtypes

import bass_rust
import concourse.bass as bass
import concourse.mybir as mybir
from concourse.ap import AP as APClass
from concourse.tile import TileContext
from concourse import bass_utils

F32 = mybir.dt.float32
F16 = mybir.dt.float16
F8 = mybir.dt.float8e4
I32 = mybir.dt.int32
PM = mybir.MatmulPerfMode

B_PER_CORE = 2
H, W, C = 360, 640, 7
GROUPS = [(0, 124, 0), (124, 124, 1), (248, 112, 2)]  # (row0, rows, variant)
WP = W + 4
NACC = 24

VERT = {0: [-2, -1, 0, 1, 2], -1: [-1, 0, 1], 1: [-1, 0, 1],
        -2: [-1, 0, 1], 2: [-1, 0, 1]}
TAPS = [(dy, dx) for dx, dys in VERT.items() for dy in dys]
DXS = [-2, -1, 0, 1, 2]
BANDW = 124
# cw16 layout: 15 conv bands (variant x dx) + identity
CW16_BLOCKS = len(GROUPS) * len(DXS) + 1
CW16_COLS = CW16_BLOCKS * BANDW


def _build_convw():
    """cw16 [128, CW16_COLS] fp16: hash-conv bands + identity (PK matmuls).
    cw8 [128, 2, 124] fp8: DoubleRow identity pair (S matmuls)."""
    rng = np.random.default_rng(20260808)
    primes = np.array([3, 7, 11, 19, 23, 31, 43, 53], dtype=np.int64)
    w = {t: int(primes[rng.integers(0, len(primes))]) * int(rng.choice([-1, 1]))
         for t in TAPS}
    wc = dict(w)
    wc[(0, 0)] = w[(0, 0)] - sum(w.values())

    w16 = np.zeros((128, CW16_BLOCKS, BANDW), dtype=np.float32)
    for vi, (r0, R, _v) in enumerate(GROUPS):
        in_r0 = max(r0 - 2, 0)
        for di, dx in enumerate(DXS):
            blk = vi * len(DXS) + di
            for j in range(R):
                for dy in VERT[dx]:
                    rr = min(max(r0 + j + dy, 0), H - 1)
                    w16[rr - in_r0, blk, j] += wc[(dy, dx)]
    for k in range(BANDW):
        w16[k, CW16_BLOCKS - 1, k] = 1.0
    assert np.abs(w16).max() <= 2048, "fp16 integer exactness"
    cw16 = np.ascontiguousarray(
        w16.reshape(128, CW16_COLS).astype(np.float16))

    w8 = np.zeros((128, 2, BANDW), dtype=np.float32)
    for k in range(BANDW):
        w8[k, 0, k] = 1.0
        w8[k, 1, k] = 1.0
    cw8 = np.ascontiguousarray(
        w8.reshape(128, 2 * BANDW).astype(ml_dtypes.float8_e4m3fn))
    return cw16, cw8


def split_multiwait_drains(nc, max_waits=1):
    """This walrus build rejects >1 sync-waits on CTRL-class instructions
    (the Tile end-of-kernel drain).  Split extra waits into preceding
    single-wait EventSemaphore instructions on the same engine."""
    fn = nc.m.functions[0]
    for bb in fn.blocks:
        for inst in list(bb.instructions):
            si = inst.sync_info
            if si is None or len(si.on_wait) <= max_waits:
                continue
            waits = list(si.on_wait)
            keep, extra = waits[:max_waits], waits[max_waits:]
            new_insts = []
            for k, wt in enumerate(extra):
                es = mybir.InstEventSemaphore(
                    name=f"{inst.name}-waitsplit-{k}", ins=[], outs=[])
                es.engine = inst.engine
                es.sync_info = bass_rust.SyncInfo(on_wait=[wt], on_update=[])
                nc.register_instruction(es, overwrite=True)
                new_insts.append(es)
            inst.sync_info = bass_rust.SyncInfo(
                on_wait=keep, on_update=list(si.on_update))
            pos = [i.name for i in bb.instructions].index(inst.name)
            for k, es in enumerate(new_insts):
                bb.instructions.insert(pos + k, es)


def _chunks(lo, hi, step=512):
    out = []
    while lo < hi:
        nxt = min(hi, (lo // step + 1) * step)
        out.append((lo, nxt))
        lo = nxt
    return out


def _pair_view(v2d, stride):
    """[P, cn] contiguous 2-D AP -> [P, 2, cn] with the two tiles `stride`
    elements apart (DoubleRow rhs)."""
    ap = [list(p) for p in v2d.ap]
    assert len(ap) == 2, ap
    return APClass(tensor=v2d.tensor, offset=v2d.offset,
                   ap=[ap[0], [stride, 2], ap[1]])


class _Dg:
    def __init__(self, gi):
        self.gi = gi
        self.r0, self.R, self.var = GROUPS[gi]
        self.in_r0 = max(self.r0 - 2, 0)
        self.in_r1 = min(self.r0 + self.R + 2, H)
        self.n_in = self.in_r1 - self.in_r0
        self.po = self.r0 - self.in_r0  # partition offset of center rows


def emit_loads(nc, io, sm, aps, dg):
    pred, target = aps["pred"], aps["target"]
    # P first: it gates exp, the critical chain.  dg0 loads per image so the
    # first image's exp starts early; later groups in one DMA (b and c merge:
    # b-stride == 7 * c-stride) to keep Pool descriptor work low.
    dg.P = io.tile([128, 2 * C * W], F16, tag="P")
    if dg.gi == 0:
        for b in range(2):
            nc.gpsimd.dma_start(
                out=dg.P[:dg.R, b * C * W:(b + 1) * C * W].rearrange(
                    "p (c w) -> p c w", c=C),
                in_=pred[b, :, dg.r0:dg.r0 + dg.R, :].rearrange(
                    "c r w -> r c w"))
    else:
        nc.gpsimd.dma_start(
            out=dg.P[:dg.R, :].rearrange("p (bc w) -> p bc w", bc=2 * C),
            in_=pred[:, :, dg.r0:dg.r0 + dg.R, :].rearrange(
                "b c r w -> r (b c) w"))
    dg.t16 = sm.tile([128, 2 * WP], F16, tag="t16")
    nc.gpsimd.dma_start(
        out=dg.t16[:dg.n_in, :].rearrange(
            "p (b wp) -> p b wp", b=2)[:, :, 2:2 + W],
        in_=target[:, dg.in_r0:dg.in_r1, :].rearrange("b r w -> r b w"))


def emit_head(nc, pools, aps, dg):
    io, sm, cvp, spp = pools
    cw16, cw8 = aps["cw16"], aps["cw8"]
    alu = mybir.AluOpType
    AF = mybir.ActivationFunctionType
    R, n_in, gi, po = dg.R, dg.n_in, dg.gi, dg.po

    # horizontal replicate pads (Pool, tiny)
    t16v = dg.t16.rearrange("p (b wp) -> p b wp", b=2)
    for b in range(2):
        nc.gpsimd.tensor_copy(t16v[:n_in, b, 0:2],
                              t16v[:n_in, b, 2:3].broadcast_to([n_in, 2]))
        nc.gpsimd.tensor_copy(t16v[:n_in, b, W + 2:W + 4],
                              t16v[:n_in, b, W + 1:W + 2].broadcast_to([n_in, 2]))

    # E = exp(P - 1) -> fp8 (dg0: per-image ops to chase the P DMA halves)
    dg.E = io.tile([128, 2 * C * W], F8, tag="E")
    splits = ((0, C * W), (C * W, 2 * C * W)) if gi == 0 else ((0, 2 * C * W),)
    for (s0, s1) in splits:
        nc.scalar.activation(dg.E[:R, s0:s1], dg.P[:R, s0:s1], AF.Exp,
                             bias=aps["neg1"][:R, 0:1], scale=1.0)

    # hash conv (PE fp16) + m_inv, per image
    cw16v = cw16.rearrange("p (blk j) -> p blk j", blk=CW16_BLOCKS)
    dg.m_inv = sm.tile([128, 2 * W], F16, tag="m_inv")
    for b in range(2):
        rr = cvp.tile([128, W], F32, tag="rr")
        for (o0, o1) in _chunks(0, W):
            cn = o1 - o0
            for di, dx in enumerate(DXS):
                blk = dg.var * len(DXS) + di
                col = b * WP + 2 + dx + o0
                nc.tensor.matmul(rr[:R, o0:o1], cw16v[:n_in, blk, :R],
                                 dg.t16[:n_in, col:col + cn],
                                 start=(di == 0), stop=(di == len(DXS) - 1))
        nc.gpsimd.tensor_scalar(
            out=dg.m_inv[:R, b * W:(b + 1) * W], in0=rr[:R, :],
            scalar1=0.0, scalar2=None, op0=alu.is_equal,
            accum_out=aps["a_mi"][:R, 2 * gi + b:2 * gi + b + 1])

    # masks (7x ts is_equal, 4x mode), then MP = M*P in place
    dg.M = io.tile([128, 2 * C * W], F16, tag="M")
    Mv = dg.M.rearrange("p (b c w) -> p b c w", b=2, c=C)
    tc16 = t16v[po:po + R, :, 2:2 + W]
    for c in range(C):
        nc.vector.tensor_scalar(out=Mv[:R, :, c, :], in0=tc16,
                                scalar1=float(c), scalar2=None, op0=alu.is_equal)
    # MP = M*P in place, one op per image so PK/d/md pipeline per image
    for b in range(2):
        s = slice(b * C * W, (b + 1) * C * W)
        nc.vector.tensor_tensor(out=dg.M[:R, s], in0=dg.M[:R, s],
                                in1=dg.P[:R, s], op=alu.mult)

    # S = sum_c E (fp8 DR pairs + single c=6); PK = sum_c MP (fp16)
    dg.S = spp.tile([128, 2 * W], F32, tag="S")
    dg.PK = spp.tile([128, 2 * W], F32, tag="PK")
    cw8v = cw8.rearrange("p (two j) -> p two j", two=2)
    id2 = cw8v[:R, :, :R]
    id8 = cw8v[:R, 0, :R]
    id16 = cw16v[:R, CW16_BLOCKS - 1, :R]
    def _emit_pk():
        for b in range(2):
            for (o0, o1) in _chunks(b * W, (b + 1) * W):
                rel0 = o0 - b * W
                cn = o1 - o0
                for c in range(C):
                    col = b * C * W + c * W + rel0
                    nc.tensor.matmul(dg.PK[:R, o0:o1], id16,
                                     dg.M[:R, col:col + cn],
                                     start=(c == 0), stop=(c == C - 1))

    def _emit_s():
        for b in range(2):
            for (o0, o1) in _chunks(b * W, (b + 1) * W):
                rel0 = o0 - b * W
                cn = o1 - o0
                for ci in range(3):
                    col = b * C * W + (2 * ci) * W + rel0
                    rhs = _pair_view(dg.E[:R, col:col + cn], W)
                    nc.tensor.matmul(dg.S[:R, o0:o1], id2, rhs,
                                     start=(ci == 0), stop=False,
                                     perf_mode=PM.DoubleRow)
                col = b * C * W + 6 * W + rel0
                nc.tensor.matmul(dg.S[:R, o0:o1], id8, dg.E[:R, col:col + cn],
                                 start=False, stop=True)

    # On the last group close the d->md tail as early as possible: MP is
    # ready before E there, so PK goes first.  Earlier groups: S first so
    # ln/exp pipelining on Act is not blocked behind MP.
    if gi == len(GROUPS) - 1:
        _emit_pk()
        _emit_s()
    else:
        _emit_s()
        _emit_pk()


def emit_tail(nc, pools, aps, dg):
    io, sm, cvp, spp = pools
    alu = mybir.AluOpType
    AF = mybir.ActivationFunctionType
    R, gi = dg.R, dg.gi

    last = gi == len(GROUPS) - 1
    lse = sm.tile([128, 2 * W], F16, tag="lse")
    nc.scalar.activation(lse[:R, :], dg.S[:R, :], AF.Ln)
    d = sm.tile([128, 2 * W], F16, tag="d")
    mdj = sm.tile([128, 2 * W], F16, tag="mdj")
    # full-width ops for early groups (fewer ops); per-image split + DVE md
    # on the last group, where the d->md chain closes the kernel
    bsplits = ((0, 2 * W),) if not last else ((0, W), (W, 2 * W))
    for bi, (s0, s1) in enumerate(bsplits):
        s = slice(s0, s1)
        nc.gpsimd.scalar_tensor_tensor(
            out=d[:R, s], in0=dg.PK[:R, s], scalar=-1.0, in1=lse[:R, s],
            op0=alu.mult, op1=alu.add,
            accum_out=aps["a_d"][:R, 2 * gi + bi:2 * gi + bi + 1])
        eng = nc.vector if last else nc.gpsimd
        eng.scalar_tensor_tensor(
            out=mdj[:R, s], in0=dg.m_inv[:R, s], scalar=0.0, in1=d[:R, s],
            op0=alu.bypass, op1=alu.mult,
            accum_out=aps["a_md"][:R, 2 * gi + bi:2 * gi + bi + 1])


def build_nc(io_bufs=2, sm_bufs=2):
    nc = bass.Bass()
    pred = nc.dram_tensor("pred", [B_PER_CORE, C, H, W], F32,
                          kind="ExternalInput")
    target = nc.dram_tensor("target", [B_PER_CORE, H, W], I32,
                            kind="ExternalInput")
    convw16 = nc.dram_tensor("convw16", [128, CW16_COLS], F16,
                             kind="ExternalInput")
    convw8 = nc.dram_tensor("convw8", [128, 2 * BANDW], F8,
                            kind="ExternalInput")
    acc_out = nc.dram_tensor("acc", [128, NACC], F32, kind="ExternalOutput")

    with TileContext(nc, pool_alloc_mode="stack") as tc:
        with (
            tc.tile_pool(name="io", bufs=io_bufs) as io,
            tc.tile_pool(name="sm", bufs=sm_bufs) as sm,
            tc.tile_pool(name="cv", bufs=1, space="PSUM") as cvp,
            tc.tile_pool(name="sp", bufs=1, space="PSUM") as spp,
            tc.tile_pool(name="const", bufs=1) as cpool,
        ):
            a_mi = cpool.tile([128, 8], F32)
            nc.vector.memset(a_mi[:, :], 0.0)
            a_d = cpool.tile([128, 8], F32)
            nc.vector.memset(a_d[:, :], 0.0)
            a_md = cpool.tile([128, 8], F32)
            nc.vector.memset(a_md[:, :], 0.0)
            neg1 = cpool.tile([128, 1], F32)
            nc.vector.memset(neg1[:, :], -1.0)
            cw16_sb = cpool.tile([128, CW16_COLS], F16)
            cw8_sb = cpool.tile([128, 2 * BANDW], F8)
            aps = {"pred": pred.ap(), "target": target.ap(),
                   "cw16": cw16_sb, "cw8": cw8_sb, "neg1": neg1,
                   "a_mi": a_mi, "a_d": a_d, "a_md": a_md}
            pools = (io, sm, cvp, spp)

            dgs = [_Dg(i) for i in range(len(GROUPS))]
            emit_loads(nc, io, sm, aps, dgs[0])
            # weights after the first P/t16 loads, split per variant block so
            # no single transfer delays P on the shared DMA engines
            nbv = len(DXS) * BANDW
            nc.sync.dma_start(out=cw8_sb[:, :], in_=convw8.ap())
            nc.sync.dma_start(out=cw16_sb[:, 15 * BANDW:],
                              in_=convw16.ap()[:, 15 * BANDW:])
            for vi in range(len(GROUPS)):
                nc.sync.dma_start(out=cw16_sb[:, vi * nbv:(vi + 1) * nbv],
                                  in_=convw16.ap()[:, vi * nbv:(vi + 1) * nbv])
            emit_loads(nc, io, sm, aps, dgs[1])
            for g in range(len(dgs)):
                emit_head(nc, pools, aps, dgs[g])
                if g + 2 < len(dgs):
                    emit_loads(nc, io, sm, aps, dgs[g + 2])
                if g > 0:
                    emit_tail(nc, pools, aps, dgs[g - 1])
            emit_tail(nc, pools, aps, dgs[-1])

            nc.sync.dma_start(out=acc_out.ap()[:, 0:8], in_=a_mi[:, :])
            nc.sync.dma_start(out=acc_out.ap()[:, 8:16], in_=a_d[:, :])
            nc.sync.dma_start(out=acc_out.ap()[:, 16:24], in_=a_md[:, :])

    split_multiwait_drains(nc)
    return nc


_CACHED = {}


def _get_nc():
    if "nc" not in _CACHED:
        _CACHED["nc"] = build_nc()
        _CACHED["convw16"], _CACHED["convw8"] = _build_convw()
    return _CACHED["nc"], _CACHED["convw16"], _CACHED["convw8"]


def combine_acc(acc_tiles):
    n_core = B_PER_CORE * H * W
    total = 0.0
    for a in acc_tiles:
        a = a.astype(np.float64)
        smi = a[:, 0:8].sum()
        sd = a[:, 8:16].sum()
        smd = a[:, 16:24].sum()
        total += 10.0 * sd - 9.0 * smd + 10.0 * n_core - 9.0 * smi
    return np.float32(total / (8 * n_core))


def kernel(pred, target):
    nc, convw16, convw8 = _get_nc()
    n_cores = 8
    in_maps = []
    for i in range(n_cores):
        in_maps.append({
            "pred": np.ascontiguousarray(pred[2 * i:2 * i + 2]),
            "target": np.ascontiguousarray(target[2 * i:2 * i + 2]),
            "convw16": convw16,
            "convw8": convw8,
        })
    res = bass_utils.run_bass_kernel_spmd(nc, in_maps,
                                          core_ids=list(range(n_cores)))
    return combine_acc([r["acc"] for r in res.results])
